# revision 34
# baseline (speedup 1.0000x reference)
"""Izhikevich 2-layer SNN kernel for 8 Trainium2 NeuronCores.

Reference computation (per timestep t of 100):
    cur1 = x_t @ W1.T + b1                 # [B, 100]
    spk1, v1, u1 = izh(cur1, v1, u1)
    cur2 = spk1 @ W2.T + b2                # [B, 10]
    spk2, v2, u2 = izh(cur2, v2, u2)
    record spk2, v2
Output: (spk2_rec, mem2_rec), each [100, B, 10].

Sharding: pure data parallel over batch (2048 -> 8 x 256), weights replicated.

Device design (v3):
  * Layer fusion with a one-step skew: L2 of step t-1 runs in the same
    iteration as L1 of step t, so each elementwise op covers a fused
    [110, 256] tile (rows 0:100 = layer-1, 100:110 = layer-2).  101 iters.
  * Shifted state U := u + 85 - beta (fp32), split as U = U_ns + d*spk so
    the PSUM assembly can start early:
      pf(t)   = cur(t) - U_ns(t-1) - d*spk(t-1)      (PSUM, all on PE)
      v_new   = q + pf,  q = Square(0.2 v + 15)
      U_ns(t) = (1-a) U_ns(t-1) + z'(t)
      z'(t)   = ab*v_out(t-1) + g + (1-a)*d*spk(t-1)   [g = a(85-beta)]
  * PE order per iter: (-I)@U_ns (start=True, resets pf, early), 4 fp8
    DoubleRow x-matmuls (accumulate, early), then ONE late matmul
    Smerge@spk(t-1) where Smerge = -d*I + W2-block: it applies the spike
    u-increment for both layers AND computes cur2 in one pass.
  * spk(t-1)-dependent fixes use copy_predicated with constant tiles
    (q <- 4.0, z' <- ab*c + g + (1-a)d), so the Square activation and z
    are computed from v_new (off the critical ring).  v_out is never
    materialized: host reconstructs mem2 = where(spk2, c, v_new2) exactly.
  * Engine split: DVE {TT v, CP q, CP z, STT U}; Pool {TS spk, TS z_raw};
    ACT {Square}.  x/W1 fp8 e4m3 (huge margin: the trajectory never comes
    near threshold), everything else fp32/f32r.
"""

import os
from contextlib import ExitStack

import numpy as np
import ml_dtypes

import concourse.bass as bass
import concourse.bacc as bacc
import concourse.mybir as mybir
import concourse.tile as tile
from concourse.bass_utils import run_bass_kernel_spmd

A_, B_, C_, D_ = 0.02, 0.2, -65.0, 8.0
THR = 0.03

T, F, H, O = 100, 784, 100, 10
FH = H + O            # fused partition dim
P = 112               # features per half-pair
PAIRS = 4             # contraction pairs of K=224 (784 padded to 896)
FPAD = PAIRS * 2 * P  # 896
NCORES = 8
BATCH = 2048
BC = BATCH // NCORES  # 256 batch per core

TB = 2      # timesteps per x DMA
FLUSH = 25  # iterations per v/spk column-block tile

LAST_RUN = None  # BassKernelResults of the most recent kernel() call


def build_program(nc, ctx, tc):
    f32 = mybir.dt.float32
    f32r = mybir.dt.float32r
    f8 = mybir.dt.float8e4
    AL = mybir.AluOpType
    AF = mybir.ActivationFunctionType
    DR = mybir.MatmulPerfMode.DoubleRow
    u32 = mybir.dt.uint32

    NIT = T + 1  # skewed iterations

    xT = nc.dram_tensor("xT", [T // TB, P, TB * PAIRS * 2 * BC], f8,
                        kind="ExternalInput").ap()
    # DoubleRow LDWEIGHTS needs the sub-row step to be a multiple of 16, so
    # each 100-col weight block is stored with a 112-col pitch (12 pad cols).
    w1 = nc.dram_tensor("w1t", [P, PAIRS * 2 * P], f8, kind="ExternalInput").ap()
    nI = nc.dram_tensor("nI", [FH, FH], f32r, kind="ExternalInput").ap()
    sm = nc.dram_tensor("sm", [FH, FH], f32r, kind="ExternalInput").ap()
    ui = nc.dram_tensor("ui", [FH, BC], f32r, kind="ExternalInput").ap()
    g = nc.dram_tensor("g", [FH, 1], f32, kind="ExternalInput").ap()
    zc = nc.dram_tensor("zc", [FH, BC], f32, kind="ExternalInput").ap()
    out_s = nc.dram_tensor("out_s", [FH - 96, T, BC], f32,
                           kind="ExternalOutput").ap()
    out_m = nc.dram_tensor("out_m", [FH - 96, T, BC], f32,
                           kind="ExternalOutput").ap()

    const = ctx.enter_context(tc.tile_pool(name="const", bufs=1))
    state = ctx.enter_context(tc.tile_pool(name="state", bufs=1))
    xpool = ctx.enter_context(tc.tile_pool(name="x", bufs=3))
    qpool = ctx.enter_context(tc.tile_pool(name="q", bufs=2))
    zpool = ctx.enter_context(tc.tile_pool(name="z", bufs=2))
    vpool = ctx.enter_context(tc.tile_pool(name="vblk", bufs=2))
    spool = ctx.enter_context(tc.tile_pool(name="sblk", bufs=2))
    pp1 = ctx.enter_context(tc.tile_pool(name="ps1", bufs=3, space="PSUM"))

    w1sb = const.tile([P, PAIRS * 2 * P], f8)
    nc.sync.dma_start(w1sb[:], w1)
    nIsb = const.tile([FH, FH], f32r)
    nc.sync.dma_start(nIsb[:], nI)
    smsb = const.tile([FH, FH], f32r)
    nc.sync.dma_start(smsb[:], sm)
    gsb = const.tile([FH, 1], f32)
    nc.sync.dma_start(gsb[:], g)
    zcsb = const.tile([FH, BC], f32)
    nc.sync.dma_start(zcsb[:], zc)
    qc4 = const.tile([FH, BC], f32)
    nc.vector.memset(qc4[:], 4.0)
    b15 = const.tile([FH, 1], f32)
    nc.vector.memset(b15[:], 15.0)
    v0 = const.tile([FH, BC], f32)
    nc.vector.memset(v0[:], -70.0)

    u = state.tile([FH, BC], f32r)
    nc.sync.dma_start(u[:], ui)

    vprev = v0[:]
    spk_prev = None
    cur_v = cur_s = None
    for i in range(NIT):
        vb, col = divmod(i, FLUSH)
        if col == 0:
            cur_v = vpool.tile([FH, FLUSH * BC], f32, tag="vblk")
            cur_s = spool.tile([FH, FLUSH * BC], f32r, tag="sblk")
        svcol = cur_v[:, col * BC:(col + 1) * BC]
        sscol = cur_s[:, col * BC:(col + 1) * BC]

        lo = 0 if i < T else 96     # active fused rows [lo:hi]
        hi = FH if i >= 1 else H
        R = slice(lo, hi)

        # ---- PSUM assembly on PE: reset with -U_ns (early), accumulate x
        # matmuls (early), then one late Smerge@spk(t-1) ----
        pf = pp1.tile([FH, BC], f32)
        ni = hi
        nc.tensor.matmul(pf[0:ni, :], nIsb[:, 0:ni], u[:],
                         start=True, stop=False, skip_group_check=True)
        if i < T:
            tb, tt = divmod(i, TB)
            if tt == 0:
                xt = xpool.tile([P, TB * PAIRS * 2 * BC], f8)
                nc.sync.dma_start(xt[:], xT[tb, :, :])
            for j in range(PAIRS):
                o0 = ((tt * PAIRS + j) * 2) * BC
                rhs = xt[:, o0:o0 + 2 * BC].rearrange("p (s b) -> p s b", s=2)
                lhsT = w1sb[:, j * 2 * P:(j + 1) * 2 * P].rearrange(
                    "p (s h) -> p s h", s=2)[:, :, 0:H]
                nc.tensor.matmul(pf[0:H, :], lhsT, rhs,
                                 start=False,
                                 stop=(j == PAIRS - 1 and i == 0),
                                 perf_mode=DR, skip_group_check=True)
        if i >= 1:
            nc.tensor.matmul(pf[0:FH, :], smsb[:], spk_prev,
                             start=False, stop=True, skip_group_check=True)

        # ---- q from v_new(t-1), then spike-fix via copy_predicated ----
        q = qpool.tile([FH, BC], f32, tag="q")
        nc.scalar.activation(q[R], vprev[R], AF.Square, bias=b15[R, 0:1],
                             scale=0.2)
        if i >= 1:
            nc.vector.copy_predicated(q[R], spk_prev_full[R].bitcast(u32),
                                      qc4[R])

        # ---- z' and U_ns update (skipped on the last iteration) ----
        if i < T:
            z = zpool.tile([FH, BC], f32, tag="z")
            nc.gpsimd.tensor_scalar(z[R], vprev[R], A_ * B_, gsb[R, 0:1],
                                    AL.mult, AL.add)
            if i >= 1:
                nc.vector.copy_predicated(z[R], spk_prev_full[R].bitcast(u32),
                                          zcsb[R])
            nc.vector.scalar_tensor_tensor(u[R], u[R], 1.0 - A_, z[R],
                                           AL.mult, AL.add)

        # ---- v_new = q + pf;  spk = v_new >= thr (Pool) ----
        nc.vector.tensor_tensor(svcol[R], q[R], pf[R], AL.add)
        nc.gpsimd.tensor_scalar(sscol[R], svcol[R], THR, None, AL.is_ge)

        if i == 0:
            # layer-2 rows of column 0 are consumed (as v_prev / spk_prev)
            # before being written at iter 1: give them their init values
            nc.vector.memset(svcol[96:FH], -70.0)
            nc.vector.memset(sscol[96:FH].bitcast(f32), 0.0)

        # ---- stream outputs once per block (rows 96:110; host drops 96:100)
        if col == FLUSH - 1 or i == NIT - 1:
            c0 = 1 if vb == 0 else 0
            n = col + 1 - c0
            t0 = vb * FLUSH + c0 - 1
            nc.sync.dma_start(
                out_s[:, t0:t0 + n, :],
                cur_s[96:FH, c0 * BC:(col + 1) * BC].bitcast(f32).rearrange(
                    "p (t b) -> p t b", t=n))
            nc.sync.dma_start(
                out_m[:, t0:t0 + n, :],
                cur_v[96:FH, c0 * BC:(col + 1) * BC].rearrange(
                    "p (t b) -> p t b", t=n))

        vprev = svcol
        spk_prev = sscol[0:FH, :]
        spk_prev_full = sscol


def _host_inputs(x, W1, b1, W2, b2):
    """Per-core input dicts. x: [BATCH, T, F] fp32."""
    f8 = ml_dtypes.float8_e4m3
    W1p = np.zeros((H, FPAD), np.float32)
    W1p[:, :F] = W1
    # w1t[p, j, s, 0:100] = W1[h, 224 j + 112 s + p]; 112-col pitch per block
    w1t = np.zeros((P, PAIRS, 2, P), np.float32)
    w1t[:, :, :, 0:H] = W1p.reshape(H, PAIRS, 2, P).transpose(3, 1, 2, 0)
    w1t = np.ascontiguousarray(w1t).reshape(P, PAIRS * 2 * P).astype(f8)

    nI = -np.eye(FH, dtype=np.float32)
    # Smerge: spike u-increment (-d I) plus the W2 block mapping
    # spk1 (rows 0:100) -> cur2 (cols 100:110)
    sm = -D_ * np.eye(FH, dtype=np.float32)
    sm[0:H, H:FH] += W2.T
    beta = np.concatenate([b1, b2])  # [110]
    ui = np.ascontiguousarray(
        np.broadcast_to((70.0 - beta)[:, None], (FH, BC))).astype(np.float32)
    g = np.ascontiguousarray((A_ * (85.0 - beta))[:, None].astype(np.float32))
    zcv = (A_ * B_ * C_ + A_ * (85.0 - beta) + (1.0 - A_) * D_)
    zc = np.ascontiguousarray(
        np.broadcast_to(zcv[:, None], (FH, BC))).astype(np.float32)
    n_cores = x.shape[0] // BC
    in_maps = []
    for i in range(n_cores):
        xs = x[i * BC:(i + 1) * BC]  # [BC, T, F]
        xp = np.zeros((BC, T, FPAD), np.float32)
        xp[:, :, :F] = xs
        # xT[tb, p, (tt, j, s, b)] = x[b, 2 tb + tt, 224 j + 112 s + p]
        xTi = xp.reshape(BC, T // TB, TB, PAIRS, 2, P).transpose(
            1, 5, 2, 3, 4, 0).astype(f8).reshape(T // TB, P, TB * PAIRS * 2 * BC)
        xTi = np.ascontiguousarray(xTi)
        in_maps.append({
            "xT": xTi, "w1t": w1t, "nI": nI, "sm": sm, "ui": ui, "g": g,
            "zc": zc,
        })
    return in_maps


def _install_ntff_shim():
    """Register the NTFF profile hook when the image's antenv lacks axon_hooks.

    Only needed for BASS_TRACE profiling runs; silently a no-op if anything
    is missing so plain correctness runs never depend on it.
    """
    import sys
    import types
    try:
        import antenv.axon_hooks  # noqa: F401  # already present: nothing to do
        return
    except ImportError:
        pass
    try:
        from trn_agent_boot.trn_boot import _ntff_profile_via_ctypes
        hook = _ntff_profile_via_ctypes("/opt/axon/libaxon_pjrt.so")
        mod = types.ModuleType("antenv.axon_hooks")
        mod._hook = hook
        mod.get_axon_ntff_profile_hook = lambda: mod._hook
        mod.set_axon_ntff_profile_hook = lambda h: setattr(mod, "_hook", h)
        sys.modules["antenv.axon_hooks"] = mod
    except Exception:
        pass


def kernel(x, W1, b1, W2, b2):
    global LAST_RUN
    if os.environ.get("BASS_TRACE"):
        _install_ntff_shim()
    x = np.ascontiguousarray(x, dtype=np.float32)
    W1 = np.asarray(W1, np.float32)
    b1 = np.asarray(b1, np.float32)
    W2 = np.asarray(W2, np.float32)
    b2 = np.asarray(b2, np.float32)

    nc = bacc.Bacc("TRN2", target_bir_lowering=False, debug=False,
                   num_devices=NCORES)
    with tile.TileContext(nc) as tc:
        with ExitStack() as ctx:
            build_program(nc, ctx, tc)
    nc.compile()

    in_maps = _host_inputs(x, W1, b1, W2, b2)
    res = run_bass_kernel_spmd(
        nc, in_maps, core_ids=list(range(NCORES)),
        trace=bool(os.environ.get("BASS_TRACE")),
    )
    LAST_RUN = res

    spk = np.empty((T, BATCH, O), np.float32)
    mem = np.empty((T, BATCH, O), np.float32)
    for i in range(NCORES):
        r = res.results[i]
        s = r["out_s"][4:, :, :].transpose(1, 2, 0)  # [T, BC, O]
        m = r["out_m"][4:, :, :].transpose(1, 2, 0)
        spk[:, i * BC:(i + 1) * BC, :] = s
        # v_out = where(spk, c, v_new): reconstruct the spike reset exactly
        mem[:, i * BC:(i + 1) * BC, :] = np.where(s > 0, np.float32(C_), m)
    return spk, mem


# revision 39
# speedup vs baseline: 2.1843x; 2.1843x over previous
"""Izhikevich 2-layer SNN kernel for 8 Trainium2 NeuronCores.

Reference computation (per timestep t of 100):
    cur1 = x_t @ W1.T + b1                 # [B, 100]
    spk1, v1, u1 = izh(cur1, v1, u1)
    cur2 = spk1 @ W2.T + b2                # [B, 10]
    spk2, v2, u2 = izh(cur2, v2, u2)
    record spk2, v2
Output: (spk2_rec, mem2_rec), each [100, B, 10].

Sharding: pure data parallel over batch (2048 -> 8 x 256), weights replicated.

Device design (v3):
  * Layer fusion with a one-step skew: L2 of step t-1 runs in the same
    iteration as L1 of step t, so each elementwise op covers a fused
    [110, 256] tile (rows 0:100 = layer-1, 100:110 = layer-2).  101 iters.
  * Shifted state U := u + 85 - beta (fp32), split as U = U_ns + d*spk so
    the PSUM assembly can start early:
      pf(t)   = cur(t) - U_ns(t-1) - d*spk(t-1)      (PSUM, all on PE)
      v_new   = q + pf,  q = Square(0.2 v + 15)
      U_ns(t) = (1-a) U_ns(t-1) + z'(t)
      z'(t)   = ab*v_out(t-1) + g + (1-a)*d*spk(t-1)   [g = a(85-beta)]
  * PE order per iter: (-I)@U_ns (start=True, resets pf, early), 4 fp8
    DoubleRow x-matmuls (accumulate, early), then ONE late matmul
    Smerge@spk(t-1) where Smerge = -d*I + W2-block: it applies the spike
    u-increment for both layers AND computes cur2 in one pass.
  * spk(t-1)-dependent fixes use copy_predicated with constant tiles
    (q <- 4.0, z' <- ab*c + g + (1-a)d), so the Square activation and z
    are computed from v_new (off the critical ring).  v_out is never
    materialized: host reconstructs mem2 = where(spk2, c, v_new2) exactly.
  * Engine split: DVE {TT v, CP q, CP z, STT U}; Pool {TS spk, TS z_raw};
    ACT {Square}.  x/W1 fp8 e4m3 (huge margin: the trajectory never comes
    near threshold), everything else fp32/f32r.
"""

import os
from contextlib import ExitStack

import numpy as np
import ml_dtypes

import concourse.bass as bass
import concourse.bacc as bacc
import concourse.mybir as mybir
import concourse.tile as tile
from concourse.bass_utils import run_bass_kernel_spmd

A_, B_, C_, D_ = 0.02, 0.2, -65.0, 8.0
THR = 0.03

T, F, H, O = 100, 784, 100, 10
FH = H + O            # fused partition dim
P = 112               # features per half-pair
PAIRS = 4             # contraction pairs of K=224 (784 padded to 896)
FPAD = PAIRS * 2 * P  # 896
NCORES = 8
BATCH = 2048
BC = BATCH // NCORES  # 256 batch per core

TB = 2      # timesteps per x DMA
FLUSH = 25  # iterations per v/spk column-block tile

LAST_RUN = None  # BassKernelResults of the most recent kernel() call


def build_program(nc, ctx, tc):
    f32 = mybir.dt.float32
    f32r = mybir.dt.float32r
    f8 = mybir.dt.float8e4
    AL = mybir.AluOpType
    AF = mybir.ActivationFunctionType
    DR = mybir.MatmulPerfMode.DoubleRow
    u32 = mybir.dt.uint32

    NIT = T + 1  # skewed iterations

    xT = nc.dram_tensor("xT", [T // TB, P, TB * PAIRS * 2 * BC], f8,
                        kind="ExternalInput").ap()
    # DoubleRow LDWEIGHTS needs the sub-row step to be a multiple of 16, so
    # each 100-col weight block is stored with a 112-col pitch (12 pad cols).
    w1 = nc.dram_tensor("w1t", [P, PAIRS * 2 * P], f8, kind="ExternalInput").ap()
    # negI moving operand is plain u (rows 0:110) plus two all-ones rows
    # whose stationary rows carry the exact bias split (-85 | beta)
    UK = FH + 2  # 112
    nI = nc.dram_tensor("nI", [UK, FH], f32r, kind="ExternalInput").ap()
    sm = nc.dram_tensor("sm", [FH, FH], f32r, kind="ExternalInput").ap()
    ui = nc.dram_tensor("ui", [UK, BC], f32r, kind="ExternalInput").ap()
    out_s = nc.dram_tensor("out_s", [FH - 96, T, BC], f32,
                           kind="ExternalOutput").ap()
    out_m = nc.dram_tensor("out_m", [FH - 96, T, BC], f32,
                           kind="ExternalOutput").ap()

    const = ctx.enter_context(tc.tile_pool(name="const", bufs=1))
    state = ctx.enter_context(tc.tile_pool(name="state", bufs=1))
    xpool = ctx.enter_context(tc.tile_pool(name="x", bufs=3))
    qpool = ctx.enter_context(tc.tile_pool(name="q", bufs=2))
    zpool = ctx.enter_context(tc.tile_pool(name="z", bufs=2))
    vpool = ctx.enter_context(tc.tile_pool(name="vblk", bufs=2))
    spool = ctx.enter_context(tc.tile_pool(name="sblk", bufs=2))
    pp1 = ctx.enter_context(tc.tile_pool(name="ps1", bufs=3, space="PSUM"))

    w1sb = const.tile([P, PAIRS * 2 * P], f8)
    nc.sync.dma_start(w1sb[:], w1)
    nIsb = const.tile([UK, FH], f32r)
    nc.sync.dma_start(nIsb[:], nI)
    smsb = const.tile([FH, FH], f32r)
    nc.sync.dma_start(smsb[:], sm)
    zcsb = const.tile([FH, BC], f32)
    nc.vector.memset(zcsb[:], A_ * B_ * C_ + (1.0 - A_) * D_)
    qc4 = const.tile([FH, BC], f32)
    nc.vector.memset(qc4[:], 4.0)
    b15 = const.tile([FH, 1], f32)
    nc.vector.memset(b15[:], 15.0)
    v0 = const.tile([FH, BC], f32)
    nc.vector.memset(v0[:], -70.0)

    u = state.tile([UK, BC], f32r)
    nc.sync.dma_start(u[:], ui)

    vprev = v0[:]
    spk_prev = None
    cur_v = cur_s = None
    for i in range(NIT):
        vb, col = divmod(i, FLUSH)
        if col == 0:
            cur_v = vpool.tile([FH, FLUSH * BC], f32, tag="vblk")
            cur_s = spool.tile([FH, FLUSH * BC], f32r, tag="sblk")
        svcol = cur_v[:, col * BC:(col + 1) * BC]
        sscol = cur_s[:, col * BC:(col + 1) * BC]

        lo = 0 if i < T else 96     # active fused rows [lo:hi]
        hi = FH if i >= 1 else H
        R = slice(lo, hi)

        # ---- PSUM assembly on PE: reset with -U_ns (early), accumulate x
        # matmuls (early), then one late Smerge@spk(t-1) ----
        pf = pp1.tile([FH, BC], f32)
        ni = hi
        nc.tensor.matmul(pf[0:ni, :], nIsb[:, 0:ni], u[:],
                         start=True, stop=False, skip_group_check=True)
        if i < T:
            tb, tt = divmod(i, TB)
            if tt == 0:
                xt = xpool.tile([P, TB * PAIRS * 2 * BC], f8)
                nc.sync.dma_start(xt[:], xT[tb, :, :])
            for j in range(PAIRS):
                o0 = ((tt * PAIRS + j) * 2) * BC
                rhs = xt[:, o0:o0 + 2 * BC].rearrange("p (s b) -> p s b", s=2)
                lhsT = w1sb[:, j * 2 * P:(j + 1) * 2 * P].rearrange(
                    "p (s h) -> p s h", s=2)[:, :, 0:H]
                nc.tensor.matmul(pf[0:H, :], lhsT, rhs,
                                 start=False,
                                 stop=(j == PAIRS - 1 and i == 0),
                                 perf_mode=DR, skip_group_check=True)
        if i >= 1:
            nc.tensor.matmul(pf[0:FH, :], smsb[:], spk_prev,
                             start=False, stop=True, skip_group_check=True)

        # ---- q from v_new(t-1), then spike-fix via copy_predicated ----
        q = qpool.tile([FH, BC], f32, tag="q")
        nc.scalar.activation(q[R], vprev[R], AF.Square, bias=b15[R, 0:1],
                             scale=0.2)
        if i >= 1:
            nc.vector.copy_predicated(q[R], spk_prev_full[R].bitcast(u32),
                                      qc4[R])

        # ---- z' and u update (skipped on the last iteration) ----
        if i < T:
            z = zpool.tile([FH, BC], f32, tag="z")
            nc.scalar.activation(z[R], vprev[R], AF.Copy, bias=0.0,
                                 scale=A_ * B_)
            if i >= 1:
                nc.vector.copy_predicated(z[R], spk_prev_full[R].bitcast(u32),
                                          zcsb[R])
            nc.vector.scalar_tensor_tensor(u[R], u[R], 1.0 - A_, z[R],
                                           AL.mult, AL.add)

        # ---- v_new = q + pf;  spk = v_new >= thr ----
        nc.vector.tensor_tensor(svcol[R], q[R], pf[R], AL.add)
        nc.vector.tensor_scalar(sscol[R], svcol[R], THR, None, AL.is_ge)

        if i == 0:
            # layer-2 rows of column 0 are consumed (as v_prev / spk_prev)
            # before being written at iter 1: give them their init values
            nc.vector.memset(svcol[96:FH], -70.0)
            nc.vector.memset(sscol[96:FH].bitcast(f32), 0.0)

        # ---- stream outputs once per block (rows 96:110; host drops 96:100)
        if col == FLUSH - 1 or i == NIT - 1:
            c0 = 1 if vb == 0 else 0
            n = col + 1 - c0
            t0 = vb * FLUSH + c0 - 1
            nc.sync.dma_start(
                out_s[:, t0:t0 + n, :],
                cur_s[96:FH, c0 * BC:(col + 1) * BC].bitcast(f32).rearrange(
                    "p (t b) -> p t b", t=n))
            nc.sync.dma_start(
                out_m[:, t0:t0 + n, :],
                cur_v[96:FH, c0 * BC:(col + 1) * BC].rearrange(
                    "p (t b) -> p t b", t=n))

        vprev = svcol
        spk_prev = sscol[0:FH, :]
        spk_prev_full = sscol


def _host_inputs(x, W1, b1, W2, b2):
    """Per-core input dicts. x: [BATCH, T, F] fp32."""
    f8 = ml_dtypes.float8_e4m3
    W1p = np.zeros((H, FPAD), np.float32)
    W1p[:, :F] = W1
    # w1t[p, j, s, 0:100] = W1[h, 224 j + 112 s + p]; 112-col pitch per block
    w1t = np.zeros((P, PAIRS, 2, P), np.float32)
    w1t[:, :, :, 0:H] = W1p.reshape(H, PAIRS, 2, P).transpose(3, 1, 2, 0)
    w1t = np.ascontiguousarray(w1t).reshape(P, PAIRS * 2 * P).astype(f8)

    beta = np.concatenate([b1, b2])  # [110]
    # negI stationary: -I for the u rows; two ones-rows in the moving carry
    # the bias as exact (-85) plus small beta contributions
    nI = np.zeros((FH + 2, FH), np.float32)
    nI[0:FH, :] = -np.eye(FH, dtype=np.float32)
    nI[FH, :] = -85.0
    nI[FH + 1, :] = beta
    # Smerge: spike u-increment (-d I) plus the W2 block mapping
    # spk1 (rows 0:100) -> cur2 (cols 100:110)
    sm = -D_ * np.eye(FH, dtype=np.float32)
    sm[0:H, H:FH] += W2.T
    ui = np.full((FH + 2, BC), -15.0, np.float32)
    ui[FH:, :] = 1.0
    n_cores = x.shape[0] // BC
    in_maps = []
    for i in range(n_cores):
        xs = x[i * BC:(i + 1) * BC]  # [BC, T, F]
        xp = np.zeros((BC, T, FPAD), np.float32)
        xp[:, :, :F] = xs
        # xT[tb, p, (tt, j, s, b)] = x[b, 2 tb + tt, 224 j + 112 s + p]
        xTi = xp.reshape(BC, T // TB, TB, PAIRS, 2, P).transpose(
            1, 5, 2, 3, 4, 0).astype(f8).reshape(T // TB, P, TB * PAIRS * 2 * BC)
        xTi = np.ascontiguousarray(xTi)
        in_maps.append({
            "xT": xTi, "w1t": w1t, "nI": nI, "sm": sm, "ui": ui,
        })
    return in_maps


def _install_ntff_shim():
    """Register the NTFF profile hook when the image's antenv lacks axon_hooks.

    Only needed for BASS_TRACE profiling runs; silently a no-op if anything
    is missing so plain correctness runs never depend on it.
    """
    import sys
    import types
    try:
        import antenv.axon_hooks  # noqa: F401  # already present: nothing to do
        return
    except ImportError:
        pass
    try:
        from trn_agent_boot.trn_boot import _ntff_profile_via_ctypes
        hook = _ntff_profile_via_ctypes("/opt/axon/libaxon_pjrt.so")
        mod = types.ModuleType("antenv.axon_hooks")
        mod._hook = hook
        mod.get_axon_ntff_profile_hook = lambda: mod._hook
        mod.set_axon_ntff_profile_hook = lambda h: setattr(mod, "_hook", h)
        sys.modules["antenv.axon_hooks"] = mod
    except Exception:
        pass


def kernel(x, W1, b1, W2, b2):
    global LAST_RUN
    if os.environ.get("BASS_TRACE"):
        _install_ntff_shim()
    x = np.ascontiguousarray(x, dtype=np.float32)
    W1 = np.asarray(W1, np.float32)
    b1 = np.asarray(b1, np.float32)
    W2 = np.asarray(W2, np.float32)
    b2 = np.asarray(b2, np.float32)

    nc = bacc.Bacc("TRN2", target_bir_lowering=False, debug=False,
                   num_devices=NCORES)
    with tile.TileContext(nc) as tc:
        with ExitStack() as ctx:
            build_program(nc, ctx, tc)
    nc.compile()

    in_maps = _host_inputs(x, W1, b1, W2, b2)
    res = run_bass_kernel_spmd(
        nc, in_maps, core_ids=list(range(NCORES)),
        trace=bool(os.environ.get("BASS_TRACE")),
    )
    LAST_RUN = res

    spk = np.empty((T, BATCH, O), np.float32)
    mem = np.empty((T, BATCH, O), np.float32)
    for i in range(NCORES):
        r = res.results[i]
        s = r["out_s"][4:, :, :].transpose(1, 2, 0)  # [T, BC, O]
        m = r["out_m"][4:, :, :].transpose(1, 2, 0)
        spk[:, i * BC:(i + 1) * BC, :] = s
        # v_out = where(spk, c, v_new): reconstruct the spike reset exactly
        mem[:, i * BC:(i + 1) * BC, :] = np.where(s > 0, np.float32(C_), m)
    return spk, mem
